# revision 6
# baseline (speedup 1.0000x reference)
"""Differential attention (DiffAttn) Trainium2 kernel, 8-core tensor-parallel.

Reference computation (per batch b, head h):
    q1,k1,q2,k2,v = x @ W*.T          (x: [B,S,D], W: [D,D], 16 heads x 128)
    a1 = softmax(q1 k1^T / sqrt(dh)); a2 = softmax(q2 k2^T / sqrt(dh))
    out = ((a1 - lam_h * a2) @ v) @ o_w.T

Sharding: tensor-parallel over heads. Core c owns heads {2c, 2c+1} (d_model
slice 256c:256c+256 of the projection outputs).  Each core computes a partial
o-projection output over its 256 input dims; the host sums the 8 partials.

Device-side layout choices:
  - x is passed pre-transposed (xt = x.T, [D, B*S]) so projections can run
    as  out.T[m, tok] = W_shard @ x.T  with the weight shard (host
    pre-transposed) as the stationary operand -> q/k tiles land in
    [head_dim(part), token(free)] layout, which feeds QK^T directly.
  - v is produced in natural [token, dim] layout (lhsT = x.T chunks) so it can
    be the stationary operand of the PV matmul.
  - probabilities are transposed in 128x128 blocks on the PE (bf16, 1cyc/row)
    to give the PV moving operand its k-on-partition layout.
  - all matmul inputs are bf16 (measured |rel err| ~5e-3 end to end vs the
    fp32 reference); PSUM accumulation is fp32; softmax stats are fp32.
"""

import math

import numpy as np
import ml_dtypes

import concourse.bacc as bacc
import concourse.bass as bass
import concourse.mybir as mybir
import concourse.tile as tile
from concourse import bass_utils
from concourse.masks import make_identity

BF16 = mybir.dt.bfloat16
F32 = mybir.dt.float32

P = 128           # partitions / head_dim / PE tile
D = 2048          # d_model
B = 2
S = 2048          # seq len
T = B * S         # 4096 tokens
NH = 16           # total heads
NHL = 2           # heads per core
MD = NHL * P      # per-core projection dim (256)
KT = D // P       # 16 contraction tiles over d_model
ST = S // P       # 16 token tiles per batch
N_CORES = 8
CHUNK = 256       # token chunk for projection x streaming
QC = 512          # q-chunk for the PV stage
SCALE = 1.0 / math.sqrt(P)

_mult = mybir.AluOpType.mult
_add = mybir.AluOpType.add


def _split_multi_waits(nc):
    """This walrus build accepts at most ONE sync-wait per instruction
    (codegen: "Too many sync wait commands").  Tile attaches one wait per
    upstream proc, so split the extras onto same-engine NOP carriers placed
    immediately before the instruction — the engine stalls on each carrier in
    turn, which is sequentially equivalent."""
    n = 0
    for bb in nc.main_func.blocks:
        out = []
        for ins in bb.instructions:
            si = getattr(ins, "sync_info", None)
            waits = list(si.on_wait) if si is not None and si.on_wait else []
            if len(waits) > 1:
                for w in waits[:-1]:
                    n += 1
                    out.append(
                        mybir.InstNoOp(
                            name=f"{ins.name}-wsplit{n}",
                            engine=ins.engine,
                            sync_info=mybir.SyncInfo(on_wait=[w], on_update=[]),
                            bass_nofuse=True,
                        )
                    )
                si.on_wait = waits[-1:]
            out.append(ins)
        bb.instructions[:] = out


def build_nc():
    nc = bass.Bass("TRN2", target_bir_lowering=False, debug=False)

    xt = nc.dram_tensor("xt", [D, T], BF16, kind="ExternalInput")
    wnames = ["wq1", "wk1", "wq2", "wk2", "wv"]
    w_d = {n: nc.dram_tensor(n, [D, MD], BF16, kind="ExternalInput") for n in wnames}
    wo_d = nc.dram_tensor("wo", [MD, D], BF16, kind="ExternalInput")
    neglam_d = nc.dram_tensor("neglam", [P, NHL], F32, kind="ExternalInput")
    out_d = nc.dram_tensor("out", [T, D], F32, kind="ExternalOutput")

    with tile.TileContext(nc) as tc:
        with (
            tc.tile_pool(name="const", bufs=1) as cpool,
            tc.tile_pool(name="proj", bufs=1) as projpool,
            tc.tile_pool(name="xchunk", bufs=2) as xpool,
            tc.tile_pool(name="attn", bufs=2) as apool,
            tc.tile_pool(name="ptp", bufs=1) as ptpool,
            tc.tile_pool(name="obufp", bufs=2) as opool,
            tc.tile_pool(name="ps_score", bufs=2, space="PSUM") as ps_score,
            tc.tile_pool(name="ps_mm", bufs=2, space="PSUM") as ps_mm,
            tc.tile_pool(name="ps_tp", bufs=2, space="PSUM") as ps_tp,
        ):
            # ---- resident constants ----
            w_sb = {}
            for n in wnames:
                t = cpool.tile([P, KT, MD], BF16, name=f"{n}_sb")
                nc.sync.dma_start(t, w_d[n].rearrange("(kt p) m -> p kt m", p=P))
                w_sb[n] = t
            wo_sb = cpool.tile([P, NHL, D], BF16)
            nc.sync.dma_start(wo_sb, wo_d.rearrange("(h p) n -> p h n", p=P))
            neglam_sb = cpool.tile([P, NHL], F32)
            nc.sync.dma_start(neglam_sb, neglam_d.ap())
            ident = cpool.tile([P, P], BF16)
            make_identity(nc, ident)

            for b in range(B):
                # ---- projections for batch b ----
                qk = {}
                for n in ["wq1", "wk1", "wq2", "wk2"]:
                    for h in range(NHL):
                        qk[(n, h)] = projpool.tile(
                            [P, S], BF16, name=f"{n}h{h}", tag=f"{n}h{h}"
                        )
                vbuf = projpool.tile([P, ST, MD], BF16, name="vbuf", tag="vbuf")

                for ci in range(S // CHUNK):
                    tok0 = b * S + ci * CHUNK
                    xc = xpool.tile([P, KT, CHUNK], BF16, name="xc", tag="xc")
                    nc.sync.dma_start(
                        xc, xt[:, tok0 : tok0 + CHUNK].rearrange("(kt p) n -> p kt n", p=P)
                    )
                    cs = slice(ci * CHUNK, (ci + 1) * CHUNK)
                    for n in ["wq1", "wk1", "wq2", "wk2"]:
                        for h in range(NHL):
                            ps = ps_mm.tile([P, QC], F32, name="psp", tag="mm")
                            for kt in range(KT):
                                nc.tensor.matmul(
                                    ps[:, :CHUNK],
                                    lhsT=w_sb[n][:, kt, h * P : (h + 1) * P],
                                    rhs=xc[:, kt, :],
                                    start=(kt == 0),
                                    stop=(kt == KT - 1),
                                )
                            nc.vector.tensor_copy(qk[(n, h)][:, cs], ps[:, :CHUNK])
                    for s4 in range(CHUNK // P):
                        ps = ps_mm.tile([P, QC], F32, name="psv", tag="mm")
                        for kt in range(KT):
                            nc.tensor.matmul(
                                ps[:, :MD],
                                lhsT=xc[:, kt, s4 * P : (s4 + 1) * P],
                                rhs=w_sb["wv"][:, kt, :],
                                start=(kt == 0),
                                stop=(kt == KT - 1),
                            )
                        nc.vector.tensor_copy(
                            vbuf[:, ci * (CHUNK // P) + s4, :], ps[:, :MD]
                        )

                # ---- attention for batch b ----
                aoT = {
                    h: projpool.tile([P, S], BF16, name=f"aoT{h}", tag=f"aoT{h}")
                    for h in range(NHL)
                }
                for h in range(NHL):
                    q1h, k1h = qk[("wq1", h)], qk[("wk1", h)]
                    q2h, k2h = qk[("wq2", h)], qk[("wk2", h)]
                    for qc in range(S // QC):
                        ptbuf = ptpool.tile([P, KT, QC], BF16, name="ptbuf", tag="pt")
                        for qt4 in range(QC // P):
                            qi = qc * QC + qt4 * P
                            ebufs, rstats = [], []
                            for si, (qh, kh) in enumerate(
                                [(q1h, k1h), (q2h, k2h)]
                            ):
                                e = apool.tile([P, S], BF16, name=f"e{si}", tag=f"e{si}")
                                ssc = apool.tile([P, 2], F32, name=f"ssc{si}", tag=f"ssc{si}")
                                for hf in range(2):
                                    ps = ps_score.tile([P, 1024], F32, name="pss", tag="score")
                                    for j in range(2):
                                        nc.tensor.matmul(
                                            ps[:, j * 512 : (j + 1) * 512],
                                            lhsT=qh[:, qi : qi + P],
                                            rhs=kh[:, hf * 1024 + j * 512 : hf * 1024 + (j + 1) * 512],
                                            start=True,
                                            stop=True,
                                        )
                                    nc.scalar.activation(
                                        e[:, hf * 1024 : (hf + 1) * 1024],
                                        ps,
                                        mybir.ActivationFunctionType.Exp,
                                        scale=SCALE,
                                        accum_out=ssc[:, hf : hf + 1],
                                    )
                                ebufs.append(e)
                                rstats.append(ssc)
                            s1 = apool.tile([P, 1], F32, name="s1", tag="s1")
                            nc.vector.tensor_add(s1, rstats[0][:, 0:1], rstats[0][:, 1:2])
                            r1 = apool.tile([P, 1], F32, name="r1", tag="r1")
                            nc.vector.reciprocal(r1, s1)
                            s2 = apool.tile([P, 1], F32, name="s2", tag="s2")
                            nc.vector.tensor_add(s2, rstats[1][:, 0:1], rstats[1][:, 1:2])
                            r2 = apool.tile([P, 1], F32, name="r2", tag="r2")
                            nc.vector.reciprocal(r2, s2)
                            r2n = apool.tile([P, 1], F32, name="r2n", tag="r2n")
                            nc.vector.tensor_mul(r2n, r2, neglam_sb[:, h : h + 1])
                            p1 = apool.tile([P, S], BF16, name="p1", tag="p1")
                            nc.vector.tensor_scalar_mul(p1, ebufs[0], r1)
                            pp = apool.tile([P, S], BF16, name="pp", tag="pp")
                            nc.vector.scalar_tensor_tensor(
                                pp, ebufs[1], r2n, p1, op0=_mult, op1=_add
                            )
                            for g in range(4):
                                pst = ps_tp.tile([P, 512], BF16, name="pst", tag="tp")
                                for t4 in range(4):
                                    kt = g * 4 + t4
                                    nc.tensor.transpose(
                                        pst[:, t4 * P : (t4 + 1) * P],
                                        pp[:, kt * P : (kt + 1) * P],
                                        ident,
                                    )
                                nc.vector.tensor_copy(
                                    ptbuf[:, g * 4 : (g + 1) * 4, qt4 * P : (qt4 + 1) * P],
                                    pst.rearrange("p (t q) -> p t q", t=4),
                                )
                        pso = ps_mm.tile([P, QC], F32, name="pso", tag="mm")
                        for kt in range(ST):
                            nc.tensor.matmul(
                                pso,
                                lhsT=vbuf[:, kt, h * P : (h + 1) * P],
                                rhs=ptbuf[:, kt, :],
                                start=(kt == 0),
                                stop=(kt == ST - 1),
                            )
                        nc.vector.tensor_copy(aoT[h][:, qc * QC : (qc + 1) * QC], pso)

                # ---- partial o-projection for batch b ----
                for tt in range(ST):
                    ob = opool.tile([P, D], F32, name="ob", tag="ob")
                    for nq in range(D // 512):
                        ps = ps_mm.tile([P, QC], F32, name="pso2", tag="mm")
                        for h in range(NHL):
                            nc.tensor.matmul(
                                ps,
                                lhsT=aoT[h][:, tt * P : (tt + 1) * P],
                                rhs=wo_sb[:, h, nq * 512 : (nq + 1) * 512],
                                start=(h == 0),
                                stop=(h == NHL - 1),
                            )
                        nc.vector.tensor_copy(ob[:, nq * 512 : (nq + 1) * 512], ps)
                    nc.sync.dma_start(
                        out_d[b * S + tt * P : b * S + (tt + 1) * P, :], ob
                    )
    _split_multi_waits(nc)
    return nc


_NC_CACHE = None


def _get_nc():
    global _NC_CACHE
    if _NC_CACHE is None:
        _NC_CACHE = build_nc()
    return _NC_CACHE


def make_in_maps(inputs):
    bf = ml_dtypes.bfloat16
    x = np.asarray(inputs["x"], np.float32)
    lam = np.asarray(inputs["lambda_param"], np.float32)
    xt = np.ascontiguousarray(x.reshape(T, D).T).astype(bf)

    in_maps = []
    for c in range(N_CORES):
        hs = slice(c * MD, (c + 1) * MD)
        m = {
            "xt": xt,
            "wq1": np.ascontiguousarray(np.asarray(inputs["q1_w"], np.float32)[hs, :].T).astype(bf),
            "wk1": np.ascontiguousarray(np.asarray(inputs["k1_w"], np.float32)[hs, :].T).astype(bf),
            "wq2": np.ascontiguousarray(np.asarray(inputs["q2_w"], np.float32)[hs, :].T).astype(bf),
            "wk2": np.ascontiguousarray(np.asarray(inputs["k2_w"], np.float32)[hs, :].T).astype(bf),
            "wv": np.ascontiguousarray(np.asarray(inputs["v_w"], np.float32)[hs, :].T).astype(bf),
            "wo": np.ascontiguousarray(np.asarray(inputs["o_w"], np.float32)[:, hs].T).astype(bf),
            "neglam": np.tile(-lam[c * NHL : (c + 1) * NHL][None, :], (P, 1)).astype(np.float32),
        }
        in_maps.append(m)
    return in_maps


def kernel(**inputs):
    in_maps = make_in_maps(inputs)
    nc = _get_nc()
    res = bass_utils.run_bass_kernel_spmd(nc, in_maps, core_ids=list(range(N_CORES)))
    acc = np.zeros((T, D), np.float64)
    for r in res.results:
        acc += np.asarray(r["out"], np.float64)
    return acc.reshape(B, S, D).astype(np.float32)


if __name__ == "__main__":
    nc = build_nc()
    print("built OK")
